# revision 3
# baseline (speedup 1.0000x reference)
"""BayesLinear sampling kernel for 8 Trainium2 NeuronCores.

Computes out[n,o] = sum_i x[n,i]*(mu_w[i,o] + sigma_w[i,o]*eps_w[n,i,o])
                    + mu_b[o] + sigma_b[o]*eps_b[n,o]
with N=4096, IN=OUT=256, data-parallel over the sample dim N (512
samples per core).  The dominant cost is streaming eps_w (1 GiB fp32)
from HBM once; the kernel is structured so every other engine stays
under the per-core DMA roofline.

Per-core layout (transposed output):
  - PSUM tiles PT[oc] of shape [128 (o), 512 (samples)] accumulate both
    the mu term (4 block matmuls, stationary mu chunk, moving xT) and
    the eps term (per sample: 4 matmuls, stationary sigma*eps chunk
    [128,128], moving xT column [128,1], output written to PSUM column s).
  - eps_w[s] ([256,256]) is DMAed as a packed [128, 512] tile
    (i-chunk c -> free columns [c*256,(c+1)*256)), multiplied in place
    by the packed sigma_w tile on the vector engine (one op per sample).
  - Bias: out_T += sigma_b[o]*eps_bT[o,s] + mu_b[o] via a fused
    tensor_scalar with per-partition scalars, then PSUM+bias add, DMA out.
Host side: shard on axis 0, transpose x/eps_b shards, exp() of the tiny
replicated params, transpose the [256,512] per-core output back.
"""

import sys

if "/opt/trn_rl_repo" not in sys.path:
    sys.path.insert(0, "/opt/trn_rl_repo")

import numpy as np

import concourse.bacc as bacc
import concourse.mybir as mybir
from concourse.bass_utils import run_bass_kernel_spmd
from concourse.tile import TileContext

N, IN, OUT = 4096, 256, 256
N_CORES = 8
B = N // N_CORES  # samples per core (512)
F32 = mybir.dt.float32

# knobs
EPS_BUFS = 12  # in-flight eps sample tiles (256 KiB each)
GPSIMD_EVERY = 3  # every k-th sample's sigma-multiply goes to GPSIMD (0 = off)

_CACHED = {}


def _build_nc():
    nc = bacc.Bacc("TRN2", target_bir_lowering=False, debug=False,
                   num_devices=N_CORES)

    eps_w = nc.declare_dram_parameter("eps_w", [B, IN, OUT], F32, isOutput=False)
    xT = nc.declare_dram_parameter("xT", [IN, B], F32, isOutput=False)
    eps_bT = nc.declare_dram_parameter("eps_bT", [OUT, B], F32, isOutput=False)
    sigp = nc.declare_dram_parameter("sigp", [128, 2 * OUT], F32, isOutput=False)
    mup = nc.declare_dram_parameter("mup", [128, 4 * 128], F32, isOutput=False)
    biasc = nc.declare_dram_parameter("biasc", [128, 4], F32, isOutput=False)
    outT = nc.declare_dram_parameter("outT", [OUT, B], F32, isOutput=True)

    with TileContext(nc) as tc:
        with (
            tc.tile_pool(name="const", bufs=1) as cpool,
            tc.tile_pool(name="psum", bufs=1, space="PSUM") as ppool,
            tc.tile_pool(name="eps", bufs=EPS_BUFS) as epool,
            tc.tile_pool(name="out", bufs=2) as opool,
        ):
            xt = [cpool.tile([128, B], F32, tag=f"xt{ic}", name=f"xt{ic}") for ic in range(2)]
            for ic in range(2):
                nc.sync.dma_start(out=xt[ic][:, :], in_=xT[ic * 128:(ic + 1) * 128, :])
            sg = cpool.tile([128, 2 * OUT], F32, tag="sg")
            nc.sync.dma_start(out=sg[:, :], in_=sigp[:, :])
            mp = cpool.tile([128, 4 * 128], F32, tag="mp")
            nc.sync.dma_start(out=mp[:, :], in_=mup[:, :])
            bc = cpool.tile([128, 4], F32, tag="bc")
            nc.sync.dma_start(out=bc[:, :], in_=biasc[:, :])
            ebt = [cpool.tile([128, B], F32, tag=f"ebt{oc}", name=f"ebt{oc}") for oc in range(2)]
            for oc in range(2):
                nc.sync.dma_start(out=ebt[oc][:, :], in_=eps_bT[oc * 128:(oc + 1) * 128, :])

            pt = [ppool.tile([128, B], F32, tag=f"pt{oc}", name=f"pt{oc}") for oc in range(2)]

            # mu term: PT[oc][o', s] = sum_ic sum_p mu[ic*128+p, oc*128+o'] * x[s, ic*128+p]
            for oc in range(2):
                for ic in range(2):
                    nc.tensor.matmul(
                        pt[oc][:, :],
                        lhsT=mp[:, (2 * ic + oc) * 128:(2 * ic + oc + 1) * 128],
                        rhs=xt[ic][:, :],
                        start=(ic == 0),
                        stop=False,
                    )

            # eps term, streamed one sample at a time
            for s in range(B):
                e = epool.tile([128, 2 * OUT], F32, tag="e")
                nc.sync.dma_start(
                    out=e[:, :].rearrange("p (c o) -> p c o", c=2),
                    in_=eps_w[s].rearrange("(c p) o -> p c o", c=2),
                )
                eng = nc.gpsimd if (GPSIMD_EVERY and s % GPSIMD_EVERY == 0) else nc.vector
                eng.tensor_mul(out=e[:, :], in0=e[:, :], in1=sg[:, :])
                for oc in range(2):
                    for ic in range(2):
                        nc.tensor.matmul(
                            pt[oc][:, s:s + 1],
                            lhsT=e[:, ic * 256 + oc * 128:ic * 256 + (oc + 1) * 128],
                            rhs=xt[ic][:, s:s + 1],
                            start=False,
                            stop=(s == B - 1 and ic == 1),
                        )

            # bias + writeback
            for oc in range(2):
                bt = opool.tile([128, B], F32, tag="bt")
                nc.vector.tensor_scalar(
                    out=bt[:, :],
                    in0=ebt[oc][:, :],
                    scalar1=bc[:, oc:oc + 1],
                    scalar2=bc[:, 2 + oc:3 + oc],
                    op0=mybir.AluOpType.mult,
                    op1=mybir.AluOpType.add,
                )
                ot = opool.tile([128, B], F32, tag="ot")
                nc.vector.tensor_add(out=ot[:, :], in0=pt[oc][:, :], in1=bt[:, :])
                nc.sync.dma_start(out=outT[oc * 128:(oc + 1) * 128, :], in_=ot[:, :])

    nc.compile()
    return nc


def _prep_in_maps(x, eps_w, eps_b, w_param1, logw_param2, b_param1, logb_param2):
    x = np.ascontiguousarray(np.asarray(x, dtype=np.float32))
    eps_w = np.asarray(eps_w, dtype=np.float32)
    eps_b = np.ascontiguousarray(np.asarray(eps_b, dtype=np.float32))
    w1 = np.asarray(w_param1, dtype=np.float32)
    lw2 = np.asarray(logw_param2, dtype=np.float32)
    b1 = np.asarray(b_param1, dtype=np.float32)
    lb2 = np.asarray(logb_param2, dtype=np.float32)

    sigw = np.exp(lw2)  # [IN, OUT]
    # sigp[p, c*256+o] = sigw[c*128+p, o]
    sigp = np.ascontiguousarray(
        sigw.reshape(2, 128, OUT).transpose(1, 0, 2).reshape(128, 2 * OUT)
    )
    # mup[p, (2ic+oc)*128+m] = w1[ic*128+p, oc*128+m]
    mup = np.ascontiguousarray(
        w1.reshape(2, 128, 2, 128).transpose(1, 0, 2, 3).reshape(128, 4 * 128)
    )
    sigb = np.exp(lb2)  # [OUT]
    biasc = np.ascontiguousarray(
        np.stack([sigb[:128], sigb[128:], b1[:128], b1[128:]], axis=1)
    )  # [128, 4]: cols = [sigb_c0, sigb_c1, b1_c0, b1_c1]

    in_maps = []
    for c in range(N_CORES):
        sl = slice(c * B, (c + 1) * B)
        in_maps.append({
            "eps_w": np.ascontiguousarray(eps_w[sl]),
            "xT": np.ascontiguousarray(x[sl].T),
            "eps_bT": np.ascontiguousarray(eps_b[sl].T),
            "sigp": sigp,
            "mup": mup,
            "biasc": biasc,
        })
    return in_maps


def kernel(x, eps_w, eps_b, w_param1, logw_param2, b_param1, logb_param2):
    if "nc" not in _CACHED:
        _CACHED["nc"] = _build_nc()
    nc = _CACHED["nc"]
    in_maps = _prep_in_maps(x, eps_w, eps_b, w_param1, logw_param2,
                            b_param1, logb_param2)
    res = run_bass_kernel_spmd(nc, in_maps, core_ids=list(range(N_CORES)))
    out = np.empty((N, OUT), dtype=np.float32)
    for c in range(N_CORES):
        out[c * B:(c + 1) * B] = res.results[c]["outT"].T
    return out


# revision 6
# speedup vs baseline: 120.6751x; 120.6751x over previous
"""BayesLinear sampling kernel for 8 Trainium2 NeuronCores.

Computes out[n,o] = sum_i x[n,i]*(mu_w[i,o] + sigma_w[i,o]*eps_w[n,i,o])
                    + mu_b[o] + sigma_b[o]*eps_b[n,o]
with N=4096, IN=OUT=256, data-parallel over the sample dim N (512
samples per core).  The dominant cost is streaming eps_w (1 GiB fp32)
from HBM once; the kernel is structured so every other engine stays
under the per-core DMA roofline (~375 us at ~358 GB/s).

Per-core layout (transposed output):
  - PSUM tiles PT[oc] of shape [128 (o), 512 (samples)] accumulate both
    the mu term (4 block matmuls, stationary mu chunk, moving xT) and
    the eps term (per sample: 4 matmuls, stationary sigma*eps chunk
    [128,128], moving xT column [128,1], output written to PSUM column s).
  - eps_w[s] ([256,256]) is DMAed as a packed [128, 512] tile
    (i-chunk c -> free columns [c*256,(c+1)*256)), multiplied in place
    by the packed sigma_w tile (vector engine, GPSIMD assists).
  - Bias: out_T += sigma_b[o]*eps_bT[o,s] + mu_b[o] via a fused
    tensor_scalar with per-partition scalars, then PSUM+bias add, DMA out.
Host side: shard on axis 0, transpose x/eps_b shards, exp() of the tiny
replicated params, transpose the [256,512] per-core output back.
"""

import sys
from contextlib import nullcontext

if "/opt/trn_rl_repo" not in sys.path:
    sys.path.insert(0, "/opt/trn_rl_repo")

import numpy as np

import concourse.bacc as bacc
import concourse.mybir as mybir
from concourse.bass_utils import run_bass_kernel_spmd
from concourse.tile import TileContext

N, IN, OUT = 4096, 256, 256
N_CORES = 8
B = N // N_CORES  # samples per core (512)
F32 = mybir.dt.float32

# knobs
EPS_BUFS = 12  # in-flight eps sample tiles (256 KiB each)
GPSIMD_EVERY = 3  # every k-th sample's sigma-multiply goes to GPSIMD (0 = off)

_CACHED = {}


def _build_nc(reps: int = 1):
    """Build the per-core bass program.  reps>1 wraps the main body in a
    Tile For_i loop that re-runs it on the same data -- used only by the
    timing harness (slope timing to cancel host/axon dispatch overhead)."""
    nc = bacc.Bacc("TRN2", target_bir_lowering=False, debug=False,
                   num_devices=N_CORES)

    eps_w = nc.declare_dram_parameter("eps_w", [B, IN, OUT], F32, isOutput=False)
    xT = nc.declare_dram_parameter("xT", [IN, B], F32, isOutput=False)
    eps_bT = nc.declare_dram_parameter("eps_bT", [OUT, B], F32, isOutput=False)
    sigp = nc.declare_dram_parameter("sigp", [128, 2 * OUT], F32, isOutput=False)
    mup = nc.declare_dram_parameter("mup", [128, 4 * 128], F32, isOutput=False)
    biasc = nc.declare_dram_parameter("biasc", [128, 4], F32, isOutput=False)
    outT = nc.declare_dram_parameter("outT", [OUT, B], F32, isOutput=True)

    with TileContext(nc) as tc:
        with (
            tc.tile_pool(name="const", bufs=1) as cpool,
            tc.tile_pool(name="psum", bufs=1, space="PSUM") as ppool,
            tc.tile_pool(name="eps", bufs=EPS_BUFS) as epool,
            tc.tile_pool(name="out", bufs=2) as opool,
        ):
            # --- constants (outside the timing loop) ---
            xt = [cpool.tile([128, B], F32, tag=f"xt{ic}", name=f"xt{ic}")
                  for ic in range(2)]
            for ic in range(2):
                nc.sync.dma_start(out=xt[ic][:, :], in_=xT[ic * 128:(ic + 1) * 128, :])
            sg = cpool.tile([128, 2 * OUT], F32, tag="sg")
            nc.sync.dma_start(out=sg[:, :], in_=sigp[:, :])
            mp = cpool.tile([128, 4 * 128], F32, tag="mp")
            nc.sync.dma_start(out=mp[:, :], in_=mup[:, :])
            bc = cpool.tile([128, 4], F32, tag="bc")
            nc.sync.dma_start(out=bc[:, :], in_=biasc[:, :])
            ebt = [cpool.tile([128, B], F32, tag=f"ebt{oc}", name=f"ebt{oc}")
                   for oc in range(2)]
            for oc in range(2):
                nc.sync.dma_start(out=ebt[oc][:, :],
                                  in_=eps_bT[oc * 128:(oc + 1) * 128, :])

            loop = tc.For_i(0, reps, 1) if reps > 1 else nullcontext()
            with loop:
                pt = [ppool.tile([128, B], F32, tag=f"pt{oc}", name=f"pt{oc}")
                      for oc in range(2)]

                # mu term:
                # PT[oc][o',s] = sum_ic sum_p mu[ic*128+p, oc*128+o'] * x[s, ic*128+p]
                for oc in range(2):
                    for ic in range(2):
                        nc.tensor.matmul(
                            pt[oc][:, :],
                            lhsT=mp[:, (2 * ic + oc) * 128:(2 * ic + oc + 1) * 128],
                            rhs=xt[ic][:, :],
                            start=(ic == 0),
                            stop=False,
                        )

                # eps term, streamed one sample at a time
                for s in range(B):
                    e = epool.tile([128, 2 * OUT], F32, tag="e")
                    nc.sync.dma_start(
                        out=e[:, :].rearrange("p (c o) -> p c o", c=2),
                        in_=eps_w[s].rearrange("(c p) o -> p c o", c=2),
                    )
                    eng = (nc.gpsimd if (GPSIMD_EVERY and s % GPSIMD_EVERY == 0)
                           else nc.vector)
                    eng.tensor_mul(out=e[:, :], in0=e[:, :], in1=sg[:, :])
                    for oc in range(2):
                        for ic in range(2):
                            nc.tensor.matmul(
                                pt[oc][:, s:s + 1],
                                lhsT=e[:, ic * 256 + oc * 128:ic * 256 + (oc + 1) * 128],
                                rhs=xt[ic][:, s:s + 1],
                                start=False,
                                stop=(s == B - 1 and ic == 1),
                            )

                # bias + writeback
                for oc in range(2):
                    bt = opool.tile([128, B], F32, tag="bt")
                    nc.vector.tensor_scalar(
                        out=bt[:, :],
                        in0=ebt[oc][:, :],
                        scalar1=bc[:, oc:oc + 1],
                        scalar2=bc[:, 2 + oc:3 + oc],
                        op0=mybir.AluOpType.mult,
                        op1=mybir.AluOpType.add,
                    )
                    ot = opool.tile([128, B], F32, tag="ot")
                    nc.vector.tensor_add(out=ot[:, :], in0=pt[oc][:, :], in1=bt[:, :])
                    nc.sync.dma_start(out=outT[oc * 128:(oc + 1) * 128, :], in_=ot[:, :])

    nc.compile()
    return nc


def _prep_in_maps(x, eps_w, eps_b, w_param1, logw_param2, b_param1, logb_param2):
    x = np.ascontiguousarray(np.asarray(x, dtype=np.float32))
    eps_w = np.asarray(eps_w, dtype=np.float32)
    eps_b = np.ascontiguousarray(np.asarray(eps_b, dtype=np.float32))
    w1 = np.asarray(w_param1, dtype=np.float32)
    lw2 = np.asarray(logw_param2, dtype=np.float32)
    b1 = np.asarray(b_param1, dtype=np.float32)
    lb2 = np.asarray(logb_param2, dtype=np.float32)

    sigw = np.exp(lw2)  # [IN, OUT]
    # sigp[p, c*256+o] = sigw[c*128+p, o]
    sigp = np.ascontiguousarray(
        sigw.reshape(2, 128, OUT).transpose(1, 0, 2).reshape(128, 2 * OUT)
    )
    # mup[p, (2ic+oc)*128+m] = w1[ic*128+p, oc*128+m]
    mup = np.ascontiguousarray(
        w1.reshape(2, 128, 2, 128).transpose(1, 0, 2, 3).reshape(128, 4 * 128)
    )
    sigb = np.exp(lb2)  # [OUT]
    biasc = np.ascontiguousarray(
        np.stack([sigb[:128], sigb[128:], b1[:128], b1[128:]], axis=1)
    )  # [128, 4]: cols = [sigb_c0, sigb_c1, b1_c0, b1_c1]

    in_maps = []
    for c in range(N_CORES):
        sl = slice(c * B, (c + 1) * B)
        in_maps.append({
            "eps_w": np.ascontiguousarray(eps_w[sl]),
            "xT": np.ascontiguousarray(x[sl].T),
            "eps_bT": np.ascontiguousarray(eps_b[sl].T),
            "sigp": sigp,
            "mup": mup,
            "biasc": biasc,
        })
    return in_maps


def kernel(x, eps_w, eps_b, w_param1, logw_param2, b_param1, logb_param2):
    if "nc" not in _CACHED:
        _CACHED["nc"] = _build_nc()
    nc = _CACHED["nc"]
    in_maps = _prep_in_maps(x, eps_w, eps_b, w_param1, logw_param2,
                            b_param1, logb_param2)
    res = run_bass_kernel_spmd(nc, in_maps, core_ids=list(range(N_CORES)))
    out = np.empty((N, OUT), dtype=np.float32)
    for c in range(N_CORES):
        out[c * B:(c + 1) * B] = res.results[c]["outT"].T
    return out
